# revision 1
# baseline (speedup 1.0000x reference)
"""Trainium2 Bass kernel for nn_ConvPixelToCapsules.

Reference computation:
  x (16, 256, 1, 20, 20) --conv W (256,1,9,9) stride 2--> votes (16,256,32,8,6,6)
  3 dynamic-routing iterations (softmax over co, weighted sum over ci,
  squash over no, agreement update) -> activation (16, 32, 8, 6, 6)

Sharding: data-parallel over batch, 2 batch elements per core on 8 cores.

Per-core layout: planes = (b, ci) = 2*256 = 512, in 4 partition-chunks of 128.
votes stored bf16 as [128 planes, (hw=36, co=32, no=8)] per chunk.
Conv = matmul with K=9 (ky) x 9 PSUM-accumulated kx steps over row-replicated
x views.  Routing: DVE multiplies; ci-reduction via N=1 matmuls with
mult-slices as lhsT (out = [hw=36 partitions, (co,no)]); iteration-1
shortcut folds the uniform route 1/32 into the reduce.
"""

import sys
import functools
import numpy as np

sys.path.insert(0, "/opt/trn_rl_repo")

import concourse.bass as bass  # noqa: E402
import concourse.tile as tile  # noqa: E402
from concourse import mybir  # noqa: E402
from concourse.bass_utils import run_bass_kernel_spmd  # noqa: E402

F32 = mybir.dt.float32
BF16 = mybir.dt.bfloat16

BS, CI, HI, WI = 16, 256, 20, 20
CO, NO, ITERS = 32, 8, 3
KH = KW = 9
HOUT = WOUT = 6
POS = HOUT * WOUT            # 36
NCORES = 8
BSH = BS // NCORES           # 2 batch elements per core
PLANES = BSH * CI            # 512
NCHUNK = PLANES // 128       # 4 chunks of 128 planes (b-major)
CONO = CO * NO               # 256
CHW = POS * CONO             # 9216 free elements per chunk of votes (hw, co, no)
HWCO = POS * CO              # 1152 logits free elements per chunk (hw, co)

Alu = mybir.AluOpType
Act = mybir.ActivationFunctionType


def ap(t, offset, dims):
    """Explicit AP on the same tensor as `t` (an AP), offset in elements."""
    return bass.AP(tensor=t.tensor, offset=t.offset + offset, ap=[list(d) for d in dims])


def _split_excess_waits(nc):
    """Walrus (enable-ldw-opt=false) allows only ONE sync-wait on DMA and
    Matmult/Ldweights pseudo-structs.  Tile sometimes emits 2+ (WAR + WAW).
    Splice a same-engine NoOp carrying the overflow waits in front."""
    import bass_rust

    exempt = set()
    nid = 0
    for f in nc.m.functions:
        for blk in f.blocks:
            out = []
            changed = False
            for ins in blk.instructions:
                si = ins.sync_info
                cap = None if type(ins).__name__ in exempt else 1
                if si is not None and cap is not None and len(si.on_wait) > cap:
                    extra = list(si.on_wait)[:-cap]
                    keep = list(si.on_wait)[-cap:]
                    for w in extra:
                        nop = bass_rust.InstNoOp(name=f"I-waitnop-{nid}")
                        nid += 1
                        nop.engine = ins.engine
                        nop.sync_info = bass_rust.SyncInfo(on_wait=[w], on_update=[])
                        out.append(nop)
                    ins.sync_info = bass_rust.SyncInfo(
                        on_wait=keep, on_update=list(si.on_update))
                    changed = True
                out.append(ins)
            if changed:
                blk.instructions = out


def build_program():
    nc = bass.Bass("TRN2", target_bir_lowering=False, debug=False)
    x_d = nc.dram_tensor("x", [PLANES, HI * WI], F32, kind="ExternalInput").ap()
    w_d = nc.dram_tensor("w", [KH * KW, CONO], F32, kind="ExternalInput").ap()
    b_d = nc.dram_tensor("b", [CONO], F32, kind="ExternalInput").ap()
    # out in [b, hw, co, no] order; host transposes to [b, co, no, h, w]
    out_d = nc.dram_tensor("out", [BSH, POS, CONO], F32, kind="ExternalOutput").ap()

    with tile.TileContext(nc) as tc:
        _emit(tc, nc, x_d, w_d, b_d, out_d)
    _split_excess_waits(nc)
    return nc


def _emit(tc, nc, x_d, w_d, b_d, out_d):
    import contextlib

    with contextlib.ExitStack() as ctx:
        persist = ctx.enter_context(tc.tile_pool(name="persist", bufs=1))
        dram = ctx.enter_context(tc.tile_pool(name="dram", bufs=2, space="DRAM"))

        # ---- persistent tiles ----
        votes = [persist.tile([128, CHW], BF16, name=f"votes{c}") for c in range(NCHUNK)]
        logits = [persist.tile([128, HWCO], F32, name=f"logits{c}") for c in range(NCHUNK)]
        bias_bc = persist.tile([POS, CONO], F32, name="bias_bc")
        ones_bf = nc.const_aps.tensor(1.0, (128, 1), BF16)  # no runtime writer
        zero_f32 = nc.const_aps.tensor(0.0, (KH, 1), F32)
        for c in range(NCHUNK):
            nc.vector.memset(logits[c][:], 0.0)

        # bias: dram [CONO] broadcast -> sbuf [POS, CONO]
        nc.sync.dma_start(out=bias_bc[:], in_=ap(b_d, 0, [[0, POS], [1, CONO]]))

        # ================= CONV =================
        # x_rows half-tile: [ky partitions(9), 128 planes, 100]:
        # element t = 40*oy' + kx + 2*ox  (oy' in 0..2), base row = ky + 6*half.
        with tc.tile_pool(name="conv_in", bufs=1) as conv_in, \
             tc.tile_pool(name="conv_ps", bufs=4, space="PSUM") as conv_ps:
            w_sb = conv_in.tile([KH, KW * CONO], F32, name="w_sb")   # [ky, (kx, com)]
            w_src = ap(w_d, 0, [[KW * CONO, KH], [CONO, KW], [1, CONO]])
            nc.sync.dma_start(out=w_sb[:], in_=w_src)
            dummy_ps = conv_ps.tile([1, 1], F32, name="dummy_ps", tag="dummy")
            nc.tensor.matmul(dummy_ps[:], ap(w_sb[:], 0, [[KW * CONO, KH], [1, 1]]),
                             zero_f32, start=True, stop=True)
            xr = conv_in.tile([KH, 128, 100], F32, name="xr")
            for c in range(NCHUNK):
                for half in range(2):
                    src = ap(
                        x_d,
                        c * 128 * 400 + half * 120,
                        [[20, KH], [400, 128], [1, 100]],
                    )
                    nc.gpsimd.dma_start(out=xr[:], in_=src)
                    dps = conv_ps.tile([1, 1], F32, name="dps", tag="dummy")
                    nc.tensor.matmul(dps[:], ap(xr[:], 0, [[12800, KH], [1, 1]]),
                                     zero_f32, start=True, stop=True)
                    for oyp in range(3):
                        oy = half * 3 + oyp
                        for ox in range(WOUT):
                            pos = oy * WOUT + ox
                            ps = conv_ps.tile([128, CONO], F32, name="cps", tag="cps")
                            for kx in range(KW):
                                lhsT = ap(xr[:], 40 * oyp + kx + 2 * ox,
                                          [[12800, KH], [100, 128]])
                                rhs = ap(w_sb[:], kx * CONO, [[KW * CONO, KH], [1, CONO]])
                                nc.tensor.matmul(
                                    ps[:], lhsT, rhs,
                                    start=(kx == 0), stop=(kx == KW - 1),
                                )
                            # evac psum [plane, (co,no)] -> votes[c][:, pos*256:+256]
                            dst = ap(votes[c][:], pos * CONO, [[CHW, 128], [1, CONO]])
                            if pos % 2 == 0:
                                nc.vector.tensor_copy(out=dst, in_=ps[:])
                            else:
                                nc.scalar.copy(out=dst, in_=ps[:])

        # ================= ROUTING =================
        rt_ps = ctx.enter_context(tc.tile_pool(name="rt_ps", bufs=2, space="PSUM"))
        work = ctx.enter_context(tc.tile_pool(name="work", bufs=1))
        small = ctx.enter_context(tc.tile_pool(name="small", bufs=1))
        for t in range(ITERS):
            if t == 0:
                red_src = votes
            else:
                # softmax over co per chunk + route*votes
                red_src = []
                for c in range(NCHUNK):
                    # logits are bounded (|logits| < ~4 here), skip max-sub
                    route = small.tile([128, HWCO], BF16, name="route", tag="route")
                    nc.scalar.activation(out=route[:], in_=logits[c][:],
                                         func=Act.Exp, scale=1.0)
                    zs = small.tile([128, POS], F32, name="zs", tag="zs")
                    r_v = ap(route[:], 0, [[HWCO, 128], [CO, POS], [1, CO]])
                    nc.vector.reduce_sum(out=zs[:], in_=r_v, axis=mybir.AxisListType.X)
                    rz = small.tile([128, POS], F32, name="rz", tag="rz")
                    nc.vector.reciprocal(out=rz[:], in_=zs[:])
                    rzb = small.tile([128, POS], BF16, name="rzb", tag="rzb")
                    nc.scalar.copy(out=rzb[:], in_=rz[:])
                    rz_b = ap(rzb[:], 0, [[POS, 128], [1, POS], [0, CO]])
                    nc.vector.tensor_tensor(route[:], route[:], rz_b, Alu.mult)
                    # mult_r = votes * route (broadcast over no)
                    mr = work.tile([128, CHW], BF16, name="mr", tag=f"mr{c % 2}")
                    r_b = ap(route[:], 0, [[HWCO, 128], [CO, POS], [1, CO], [0, NO]])
                    nc.vector.tensor_tensor(mr[:], votes[c][:], r_b, Alu.mult)
                    red_src.append(mr)

            # preact[b][hw, (co,no)] = sum_ci red_src: N=1 matmuls,
            # lhsT = red_src slice [128, hw(36) strided], rhs = ones.
            pre_ps = []
            for b in range(BSH):
                pp = rt_ps.tile([POS, CONO], F32, name="pp", tag=f"pp{b}")
                pre_ps.append(pp)
                for cono in range(CONO):
                    for k, c in enumerate((2 * b, 2 * b + 1)):
                        lhsT = ap(red_src[c][:], cono, [[CHW, 128], [CONO, POS]])
                        nc.tensor.matmul(
                            pp[:, cono:cono + 1], lhsT, ones_bf,
                            start=(k == 0), stop=(k == 1),
                        )

            acts = []
            for b in range(BSH):
                pp = pre_ps[b]
                # preb = pp * (1/CO for iter 0) + bias
                preb = small.tile([POS, CONO], F32, name="preb", tag=f"preb{b}")
                nc.vector.scalar_tensor_tensor(
                    preb[:], pp[:], (1.0 / CO) if t == 0 else 1.0, bias_bc[:],
                    Alu.mult, Alu.add)
                # squash over no
                sq = small.tile([POS, CONO], F32, name="sq", tag=f"sq{b}")
                nc.scalar.activation(out=sq[:], in_=preb[:], func=Act.Square, scale=1.0)
                s2 = small.tile([POS, CO], F32, name="s2", tag=f"s2{b}")
                sq_v = ap(sq[:], 0, [[CONO, POS], [NO, CO], [1, NO]])
                nc.vector.reduce_sum(out=s2[:], in_=sq_v, axis=mybir.AxisListType.X)
                nrm = small.tile([POS, CO], F32, name="nrm", tag=f"nrm{b}")
                nc.scalar.activation(out=nrm[:], in_=s2[:], func=Act.Sqrt, scale=1.0)
                d1 = small.tile([POS, CO], F32, name="d1", tag=f"d1{b}")
                nc.vector.tensor_scalar_add(d1[:], s2[:], 1.0)
                r1 = small.tile([POS, CO], F32, name="r1", tag=f"r1{b}")
                nc.vector.reciprocal(out=r1[:], in_=d1[:])
                fac = small.tile([POS, CO], F32, name="fac", tag=f"fac{b}")
                nc.vector.tensor_tensor(fac[:], nrm[:], r1[:], Alu.mult)
                fac_b = ap(fac[:], 0, [[CO, POS], [1, CO], [0, NO]])
                if t == ITERS - 1:
                    af = small.tile([POS, CONO], F32, name="af", tag=f"af{b}")
                    nc.vector.tensor_tensor(af[:], preb[:], fac_b, Alu.mult)
                    nc.sync.dma_start(
                        out=ap(out_d, b * POS * CONO, [[CONO, POS], [1, CONO]]),
                        in_=af[:],
                    )
                else:
                    ab = small.tile([POS, CONO], BF16, name="ab", tag=f"ab{b}")
                    nc.vector.tensor_tensor(ab[:], preb[:], fac_b, Alu.mult)
                    acts.append(ab)

            if t == ITERS - 1:
                break

            # distances + logits update
            for b in range(BSH):
                # act [POS part, CONO] -> dram bounce -> bcast [128, (hw,co,no)]
                adr = dram.tile([POS, CONO], BF16, name="adr", tag="adr")
                nc.gpsimd.dma_start(out=adr[:], in_=acts[b][:])
                abc = work.tile([128, CHW], BF16, name="abc", tag=f"abc{b}")
                nc.gpsimd.dma_start(out=abc[:], in_=ap(adr[:], 0, [[0, 128], [1, CHW]]))
                for c in (2 * b, 2 * b + 1):
                    md = work.tile([128, CHW], BF16, name="md", tag="md")
                    nc.vector.tensor_tensor(md[:], votes[c][:], abc[:], Alu.mult)
                    dist = small.tile([128, HWCO], F32, name="dist", tag="dist")
                    md_v = ap(md[:], 0, [[CHW, 128], [NO, HWCO], [1, NO]])
                    nc.vector.reduce_sum(out=dist[:], in_=md_v, axis=mybir.AxisListType.X)
                    nc.vector.tensor_tensor(logits[c][:], logits[c][:], dist[:], Alu.add)


@functools.cache
def _program():
    return build_program()


def kernel(x, W, bias, **_ignored):
    x = np.asarray(x, dtype=np.float32)
    W = np.asarray(W, dtype=np.float32)
    bias = np.asarray(bias, dtype=np.float32)
    nc = _program()
    w_flat = np.ascontiguousarray(W.reshape(CONO, KH * KW).T)  # [81, 256] k-major
    b_flat = np.ascontiguousarray(bias.reshape(CONO))
    in_maps = []
    for i in range(NCORES):
        xs = x[i * BSH:(i + 1) * BSH].reshape(PLANES, HI * WI)
        in_maps.append({
            "x": np.ascontiguousarray(xs),
            "w": w_flat,
            "b": b_flat,
        })
    res = run_bass_kernel_spmd(nc, in_maps, list(range(NCORES)))
    outs = []
    for i in range(NCORES):
        o = res.results[i]["out"].reshape(BSH, POS, CO, NO)
        outs.append(np.transpose(o, (0, 2, 3, 1)).reshape(BSH, CO, NO, HOUT, WOUT))
    return np.ascontiguousarray(np.concatenate(outs, axis=0))


if __name__ == "__main__":
    xs = np.random.randn(BS, CI, 1, HI, WI).astype(np.float32)
    ws = (np.random.randn(CONO, 1, KH, KW) * 0.05).astype(np.float32)
    bs_ = (np.random.randn(CO, NO, 1, 1) * 0.01).astype(np.float32)
    y = kernel(xs, ws, bs_, quantization_bits=8, quantization_bits_routing=8)
    print(y.shape, y.dtype)



# revision 5
# speedup vs baseline: 1.7019x; 1.7019x over previous
"""Trainium2 Bass kernel for nn_ConvPixelToCapsules.

Reference computation:
  x (16, 256, 1, 20, 20) --conv W (256,1,9,9) stride 2--> votes (16,256,32,8,6,6)
  3 dynamic-routing iterations (softmax over co, weighted sum over ci,
  squash over no, agreement update) -> activation (16, 32, 8, 6, 6)

Sharding: data-parallel over batch, 2 batch elements per core on 8 cores.

Per-core layout: planes = (b, ci) = 2*256 = 512, in 4 partition-chunks of 128.
votes stored bf16 as [128 planes, (hw=36, co=32, no=8)] per chunk.
Conv = matmul with K=9 (ky) x 9 PSUM-accumulated kx steps over row-replicated
x views.  Routing: DVE multiplies; ci-reduction via N=1 matmuls with
mult-slices as lhsT (out = [hw=36 partitions, (co,no)]); iteration-1
shortcut folds the uniform route 1/32 into the reduce.
"""

import sys
import functools
import numpy as np

sys.path.insert(0, "/opt/trn_rl_repo")

import concourse.bass as bass  # noqa: E402
import concourse.tile as tile  # noqa: E402
from concourse import mybir  # noqa: E402
from concourse.bass_utils import run_bass_kernel_spmd  # noqa: E402

F32 = mybir.dt.float32
BF16 = mybir.dt.bfloat16

BS, CI, HI, WI = 16, 256, 20, 20
CO, NO, ITERS = 32, 8, 3
KH = KW = 9
HOUT = WOUT = 6
POS = HOUT * WOUT            # 36
NCORES = 8
BSH = BS // NCORES           # 2 batch elements per core
PLANES = BSH * CI            # 512
NCHUNK = PLANES // 128       # 4 chunks of 128 planes (b-major)
CONO = CO * NO               # 256
CHW = POS * CONO             # 9216 free elements per chunk of votes (hw, co, no)
HWCO = POS * CO              # 1152 logits free elements per chunk (hw, co)

Alu = mybir.AluOpType
Act = mybir.ActivationFunctionType


def ap(t, offset, dims):
    """Explicit AP on the same tensor as `t` (an AP), offset in elements."""
    return bass.AP(tensor=t.tensor, offset=t.offset + offset, ap=[list(d) for d in dims])


def _split_excess_waits(nc):
    """Walrus (enable-ldw-opt=false) allows only ONE sync-wait on DMA and
    Matmult/Ldweights pseudo-structs.  Tile sometimes emits 2+ (WAR + WAW).
    Splice a same-engine NoOp carrying the overflow waits in front."""
    import bass_rust

    exempt = set()
    nid = 0
    for f in nc.m.functions:
        for blk in f.blocks:
            out = []
            changed = False
            for ins in blk.instructions:
                si = ins.sync_info
                cap = None if type(ins).__name__ in exempt else 1
                if si is not None and cap is not None and len(si.on_wait) > cap:
                    extra = list(si.on_wait)[:-cap]
                    keep = list(si.on_wait)[-cap:]
                    for w in extra:
                        nop = bass_rust.InstNoOp(name=f"I-waitnop-{nid}")
                        nid += 1
                        nop.engine = ins.engine
                        nop.sync_info = bass_rust.SyncInfo(on_wait=[w], on_update=[])
                        out.append(nop)
                    ins.sync_info = bass_rust.SyncInfo(
                        on_wait=keep, on_update=list(si.on_update))
                    changed = True
                out.append(ins)
            if changed:
                blk.instructions = out


def build_program():
    nc = bass.Bass("TRN2", target_bir_lowering=False, debug=False)
    x_d = nc.dram_tensor("x", [PLANES, HI * WI], F32, kind="ExternalInput").ap()
    w_d = nc.dram_tensor("w", [KH * KW, CONO], F32, kind="ExternalInput").ap()
    b_d = nc.dram_tensor("b", [CONO], F32, kind="ExternalInput").ap()
    # out in [b, hw, co, no] order; host transposes to [b, co, no, h, w]
    out_d = nc.dram_tensor("out", [BSH, POS, CONO], F32, kind="ExternalOutput").ap()

    with tile.TileContext(nc) as tc:
        _emit(tc, nc, x_d, w_d, b_d, out_d)
    _split_excess_waits(nc)
    return nc


def _emit(tc, nc, x_d, w_d, b_d, out_d):
    import contextlib

    with contextlib.ExitStack() as ctx:
        persist = ctx.enter_context(tc.tile_pool(name="persist", bufs=1))
        dram = ctx.enter_context(tc.tile_pool(name="dram", bufs=2, space="DRAM"))

        # ---- persistent tiles ----
        votes = [persist.tile([128, CHW], BF16, name=f"votes{c}") for c in range(NCHUNK)]
        logits = [persist.tile([128, HWCO], F32, name=f"logits{c}") for c in range(NCHUNK)]
        bias_bc = persist.tile([POS, CONO], F32, name="bias_bc")
        ones_bf = nc.const_aps.tensor(1.0, (128, 1), BF16)  # no runtime writer
        zero_f32 = nc.const_aps.tensor(0.0, (KH, 1), F32)
        for c in range(NCHUNK):
            nc.vector.memset(logits[c][:], 0.0)

        # bias: dram [CONO] broadcast -> sbuf [POS, CONO]
        nc.sync.dma_start(out=bias_bc[:], in_=ap(b_d, 0, [[0, POS], [1, CONO]]))

        # ================= CONV =================
        # K=81 im2col matmuls: per chunk-half build im2col [81 taps, (oyp, plane,
        # c=2*ox window 16)] bf16 via SWDGE cast-DMA from xr [9 ky, 128 planes, 100],
        # then one matmul per output position: lhsT = im2col slice [81, 128 planes],
        # rhs = W [81, 256] -> psum [128 planes, 256 cono].
        with tc.tile_pool(name="conv_in", bufs=2) as conv_in, \
             tc.tile_pool(name="conv_ps", bufs=4, space="PSUM") as conv_ps, \
             tc.tile_pool(name="conv_w", bufs=1) as conv_w:
            w_f32 = conv_w.tile([KH * KW, CONO], F32, name="w_f32")
            nc.sync.dma_start(out=w_f32[:], in_=w_d)  # [81, 256] (ky,kx)-major
            w_bf = conv_w.tile([KH * KW, CONO], BF16, name="w_bf")
            nc.vector.tensor_copy(out=w_bf[:], in_=w_f32[:])
            for c in range(NCHUNK):
                for half in range(2):
                    xr = conv_in.tile([KH, 128 * 100], BF16, name="xr", tag="xr")
                    src = ap(x_d, c * 128 * 400 + half * 120,
                             [[20, KH], [400, 128], [1, 100]])
                    nc.gpsimd.dma_start(out=xr[:], in_=src)
                    # im partitions kx-major: p = kx*9 + ky (w rows match on host);
                    # per-plane window of 96 cols, c0 = 40*oyp + 2*ox in [0, 90].
                    im = conv_in.tile([81, 128 * 96], BF16, name="im", tag="im")
                    for kx in range(KW):
                        im_dst = ap(im[:], kx * 9 * 128 * 96,
                                    [[128 * 96, 9], [96, 128], [1, 91]])
                        im_src = ap(xr[:], kx, [[12800, KH], [100, 128], [1, 91]])
                        nc.gpsimd.dma_start(out=im_dst, in_=im_src)
                    for oyp in range(3):
                        oy = half * 3 + oyp
                        for oxp in range(WOUT // 2):
                            pos = oy * WOUT + 2 * oxp
                            ps = conv_ps.tile([128, 2 * CONO], F32, name="cps", tag="cps")
                            for j in range(2):
                                lhsT = ap(im[:], 40 * oyp + 2 * (2 * oxp + j),
                                          [[128 * 96, 81], [96, 128]])
                                nc.tensor.matmul(ps[:, j * CONO:(j + 1) * CONO],
                                                 lhsT, w_bf[:], start=True, stop=True)
                            # evac psum [plane, (2 pos, co, no)] -> votes
                            dst = ap(votes[c][:], pos * CONO, [[CHW, 128], [1, 2 * CONO]])
                            if oxp % 2 == 0:
                                nc.vector.tensor_copy(out=dst, in_=ps[:])
                            else:
                                nc.scalar.copy(out=dst, in_=ps[:])

        # ================= ROUTING =================
        rt_ps = ctx.enter_context(tc.tile_pool(name="rt_ps", bufs=2, space="PSUM"))
        work = ctx.enter_context(tc.tile_pool(name="work", bufs=1))
        small = ctx.enter_context(tc.tile_pool(name="small", bufs=1))
        for t in range(ITERS):
            if t == 0:
                red_src = votes
            else:
                # softmax over co per chunk + route*votes
                red_src = []
                for c in range(NCHUNK):
                    # logits are bounded (|logits| < ~4 here), skip max-sub
                    route = small.tile([128, HWCO], BF16, name="route", tag="route")
                    nc.scalar.activation(out=route[:], in_=logits[c][:],
                                         func=Act.Exp, scale=1.0)
                    zs = small.tile([128, POS], F32, name="zs", tag="zs")
                    r_v = ap(route[:], 0, [[HWCO, 128], [CO, POS], [1, CO]])
                    nc.vector.reduce_sum(out=zs[:], in_=r_v, axis=mybir.AxisListType.X)
                    rz = small.tile([128, POS], F32, name="rz", tag="rz")
                    nc.vector.reciprocal(out=rz[:], in_=zs[:])
                    rzb = small.tile([128, POS], BF16, name="rzb", tag="rzb")
                    nc.scalar.copy(out=rzb[:], in_=rz[:])
                    rz_b = ap(rzb[:], 0, [[POS, 128], [1, POS], [0, CO]])
                    nc.vector.tensor_tensor(route[:], route[:], rz_b, Alu.mult)
                    # mult_r = votes * route (broadcast over no)
                    mr = work.tile([128, CHW], BF16, name="mr", tag=f"mr{c % 2}")
                    r_b = ap(route[:], 0, [[HWCO, 128], [CO, POS], [1, CO], [0, NO]])
                    nc.vector.tensor_tensor(mr[:], votes[c][:], r_b, Alu.mult)
                    red_src.append(mr)

            # preact[b][hw, (co,no)] = sum_ci red_src: N=1 matmuls,
            # lhsT = red_src slice [128, hw(36) strided], rhs = ones.
            pre_ps = []
            for b in range(BSH):
                pp = rt_ps.tile([POS, CONO], F32, name="pp", tag=f"pp{b}")
                pre_ps.append(pp)
                for cono in range(CONO):
                    for k, c in enumerate((2 * b, 2 * b + 1)):
                        lhsT = ap(red_src[c][:], cono, [[CHW, 128], [CONO, POS]])
                        nc.tensor.matmul(
                            pp[:, cono:cono + 1], lhsT, ones_bf,
                            start=(k == 0), stop=(k == 1),
                        )

            acts = []
            for b in range(BSH):
                pp = pre_ps[b]
                # preb = pp * (1/CO for iter 0) + bias
                preb = small.tile([POS, CONO], F32, name="preb", tag=f"preb{b}")
                nc.vector.scalar_tensor_tensor(
                    preb[:], pp[:], (1.0 / CO) if t == 0 else 1.0, bias_bc[:],
                    Alu.mult, Alu.add)
                # squash over no
                sq = small.tile([POS, CONO], F32, name="sq", tag=f"sq{b}")
                nc.scalar.activation(out=sq[:], in_=preb[:], func=Act.Square, scale=1.0)
                s2 = small.tile([POS, CO], F32, name="s2", tag=f"s2{b}")
                sq_v = ap(sq[:], 0, [[CONO, POS], [NO, CO], [1, NO]])
                nc.vector.reduce_sum(out=s2[:], in_=sq_v, axis=mybir.AxisListType.X)
                nrm = small.tile([POS, CO], F32, name="nrm", tag=f"nrm{b}")
                nc.scalar.activation(out=nrm[:], in_=s2[:], func=Act.Sqrt, scale=1.0)
                d1 = small.tile([POS, CO], F32, name="d1", tag=f"d1{b}")
                nc.vector.tensor_scalar_add(d1[:], s2[:], 1.0)
                r1 = small.tile([POS, CO], F32, name="r1", tag=f"r1{b}")
                nc.vector.reciprocal(out=r1[:], in_=d1[:])
                fac = small.tile([POS, CO], F32, name="fac", tag=f"fac{b}")
                nc.vector.tensor_tensor(fac[:], nrm[:], r1[:], Alu.mult)
                fac_b = ap(fac[:], 0, [[CO, POS], [1, CO], [0, NO]])
                if t == ITERS - 1:
                    af = small.tile([POS, CONO], F32, name="af", tag=f"af{b}")
                    nc.vector.tensor_tensor(af[:], preb[:], fac_b, Alu.mult)
                    nc.sync.dma_start(
                        out=ap(out_d, b * POS * CONO, [[CONO, POS], [1, CONO]]),
                        in_=af[:],
                    )
                else:
                    ab = small.tile([POS, CONO], BF16, name="ab", tag=f"ab{b}")
                    nc.vector.tensor_tensor(ab[:], preb[:], fac_b, Alu.mult)
                    acts.append(ab)

            if t == ITERS - 1:
                break

            # distances + logits update
            for b in range(BSH):
                # act [POS part, CONO] -> dram bounce -> bcast [128, (hw,co,no)]
                adr = dram.tile([POS, CONO], BF16, name="adr", tag="adr")
                nc.gpsimd.dma_start(out=adr[:], in_=acts[b][:])
                abc = work.tile([128, CHW], BF16, name="abc", tag=f"abc{b}")
                nc.gpsimd.dma_start(out=abc[:], in_=ap(adr[:], 0, [[0, 128], [1, CHW]]))
                for c in (2 * b, 2 * b + 1):
                    md = work.tile([128, CHW], BF16, name="md", tag="md")
                    nc.vector.tensor_tensor(md[:], votes[c][:], abc[:], Alu.mult)
                    dist = small.tile([128, HWCO], F32, name="dist", tag="dist")
                    md_v = ap(md[:], 0, [[CHW, 128], [NO, HWCO], [1, NO]])
                    nc.vector.reduce_sum(out=dist[:], in_=md_v, axis=mybir.AxisListType.X)
                    nc.vector.tensor_tensor(logits[c][:], logits[c][:], dist[:], Alu.add)


@functools.cache
def _program():
    return build_program()


def kernel(x, W, bias, **_ignored):
    x = np.asarray(x, dtype=np.float32)
    W = np.asarray(W, dtype=np.float32)
    bias = np.asarray(bias, dtype=np.float32)
    nc = _program()
    # [81, 256], rows kx-major: r = kx*9 + ky
    w_flat = np.ascontiguousarray(
        W.reshape(CONO, KH, KW).transpose(2, 1, 0).reshape(KH * KW, CONO))
    b_flat = np.ascontiguousarray(bias.reshape(CONO))
    in_maps = []
    for i in range(NCORES):
        xs = x[i * BSH:(i + 1) * BSH].reshape(PLANES, HI * WI)
        in_maps.append({
            "x": np.ascontiguousarray(xs),
            "w": w_flat,
            "b": b_flat,
        })
    res = run_bass_kernel_spmd(nc, in_maps, list(range(NCORES)))
    outs = []
    for i in range(NCORES):
        o = res.results[i]["out"].reshape(BSH, POS, CO, NO)
        outs.append(np.transpose(o, (0, 2, 3, 1)).reshape(BSH, CO, NO, HOUT, WOUT))
    return np.ascontiguousarray(np.concatenate(outs, axis=0))


if __name__ == "__main__":
    xs = np.random.randn(BS, CI, 1, HI, WI).astype(np.float32)
    ws = (np.random.randn(CONO, 1, KH, KW) * 0.05).astype(np.float32)
    bs_ = (np.random.randn(CO, NO, 1, 1) * 0.01).astype(np.float32)
    y = kernel(xs, ws, bs_, quantization_bits=8, quantization_bits_routing=8)
    print(y.shape, y.dtype)



# revision 6
# speedup vs baseline: 2.0986x; 1.2331x over previous
"""Trainium2 Bass kernel for nn_ConvPixelToCapsules.

Reference computation:
  x (16, 256, 1, 20, 20) --conv W (256,1,9,9) stride 2--> votes (16,256,32,8,6,6)
  3 dynamic-routing iterations (softmax over co, weighted sum over ci,
  squash over no, agreement update) -> activation (16, 32, 8, 6, 6)

Sharding: data-parallel over batch, 2 batch elements per core on 8 cores.

Per-core layout: planes = (b, ci) = 2*256 = 512, in 4 partition-chunks of 128.
votes stored bf16 as [128 planes, (hw=36, co=32, no=8)] per chunk.
Conv = matmul with K=9 (ky) x 9 PSUM-accumulated kx steps over row-replicated
x views.  Routing: DVE multiplies; ci-reduction via N=1 matmuls with
mult-slices as lhsT (out = [hw=36 partitions, (co,no)]); iteration-1
shortcut folds the uniform route 1/32 into the reduce.
"""

import sys
import functools
import numpy as np

sys.path.insert(0, "/opt/trn_rl_repo")

import concourse.bass as bass  # noqa: E402
import concourse.tile as tile  # noqa: E402
from concourse import mybir  # noqa: E402
from concourse.bass_utils import run_bass_kernel_spmd  # noqa: E402

F32 = mybir.dt.float32
BF16 = mybir.dt.bfloat16

BS, CI, HI, WI = 16, 256, 20, 20
CO, NO, ITERS = 32, 8, 3
KH = KW = 9
HOUT = WOUT = 6
POS = HOUT * WOUT            # 36
NCORES = 8
BSH = BS // NCORES           # 2 batch elements per core
PLANES = BSH * CI            # 512
NCHUNK = PLANES // 128       # 4 chunks of 128 planes (b-major)
CONO = CO * NO               # 256
CHW = POS * CONO             # 9216 free elements per chunk of votes (hw, co, no)
HWCO = POS * CO              # 1152 logits free elements per chunk (hw, co)

Alu = mybir.AluOpType
Act = mybir.ActivationFunctionType


def ap(t, offset, dims):
    """Explicit AP on the same tensor as `t` (an AP), offset in elements."""
    return bass.AP(tensor=t.tensor, offset=t.offset + offset, ap=[list(d) for d in dims])


def _split_excess_waits(nc):
    """Walrus (enable-ldw-opt=false) allows only ONE sync-wait on DMA and
    Matmult/Ldweights pseudo-structs.  Tile sometimes emits 2+ (WAR + WAW).
    Splice a same-engine NoOp carrying the overflow waits in front."""
    import bass_rust

    exempt = set()
    nid = 0
    for f in nc.m.functions:
        for blk in f.blocks:
            out = []
            changed = False
            for ins in blk.instructions:
                si = ins.sync_info
                cap = None if type(ins).__name__ in exempt else 1
                if si is not None and cap is not None and len(si.on_wait) > cap:
                    extra = list(si.on_wait)[:-cap]
                    keep = list(si.on_wait)[-cap:]
                    for w in extra:
                        nop = bass_rust.InstNoOp(name=f"I-waitnop-{nid}")
                        nid += 1
                        nop.engine = ins.engine
                        nop.sync_info = bass_rust.SyncInfo(on_wait=[w], on_update=[])
                        out.append(nop)
                    ins.sync_info = bass_rust.SyncInfo(
                        on_wait=keep, on_update=list(si.on_update))
                    changed = True
                out.append(ins)
            if changed:
                blk.instructions = out


def build_program():
    nc = bass.Bass("TRN2", target_bir_lowering=False, debug=False)
    x_d = nc.dram_tensor("x", [PLANES, HI * WI], F32, kind="ExternalInput").ap()
    w_d = nc.dram_tensor("w", [KH * KW, CONO], F32, kind="ExternalInput").ap()
    b_d = nc.dram_tensor("b", [CONO], F32, kind="ExternalInput").ap()
    # out in [b, hw, co, no] order; host transposes to [b, co, no, h, w]
    out_d = nc.dram_tensor("out", [BSH, POS, CONO], F32, kind="ExternalOutput").ap()

    with tile.TileContext(nc) as tc:
        _emit(tc, nc, x_d, w_d, b_d, out_d)
    _split_excess_waits(nc)
    return nc


def _emit(tc, nc, x_d, w_d, b_d, out_d):
    import contextlib

    with contextlib.ExitStack() as ctx:
        persist = ctx.enter_context(tc.tile_pool(name="persist", bufs=1))
        dram = ctx.enter_context(tc.tile_pool(name="dram", bufs=2, space="DRAM"))

        # ---- persistent tiles ----
        votes = [persist.tile([128, CHW], BF16, name=f"votes{c}") for c in range(NCHUNK)]
        logits = [persist.tile([128, HWCO], F32, name=f"logits{c}") for c in range(NCHUNK)]
        bias_bc = persist.tile([POS, CONO], F32, name="bias_bc")
        ones_bf = nc.const_aps.tensor(1.0, (128, 1), BF16)  # no runtime writer
        zero_f32 = nc.const_aps.tensor(0.0, (KH, 1), F32)
        for c in range(NCHUNK):
            nc.vector.memset(logits[c][:], 0.0)

        # bias: dram [CONO] broadcast -> sbuf [POS, CONO]
        nc.sync.dma_start(out=bias_bc[:], in_=ap(b_d, 0, [[0, POS], [1, CONO]]))

        # ================= CONV =================
        # K=81 im2col matmuls: per chunk-half build im2col [81 taps, (oyp, plane,
        # c=2*ox window 16)] bf16 via SWDGE cast-DMA from xr [9 ky, 128 planes, 100],
        # then one matmul per output position: lhsT = im2col slice [81, 128 planes],
        # rhs = W [81, 256] -> psum [128 planes, 256 cono].
        with tc.tile_pool(name="conv_in", bufs=2) as conv_in, \
             tc.tile_pool(name="conv_ps", bufs=4, space="PSUM") as conv_ps, \
             tc.tile_pool(name="conv_w", bufs=1) as conv_w:
            w_f32 = conv_w.tile([KH * KW, CONO], F32, name="w_f32")
            nc.sync.dma_start(out=w_f32[:], in_=w_d)  # [81, 256] (ky,kx)-major
            w_bf = conv_w.tile([KH * KW, CONO], BF16, name="w_bf")
            nc.vector.tensor_copy(out=w_bf[:], in_=w_f32[:])
            for c in range(NCHUNK):
                for half in range(2):
                    xr = conv_in.tile([KH, 128 * 100], BF16, name="xr", tag="xr")
                    src = ap(x_d, c * 128 * 400 + half * 120,
                             [[20, KH], [400, 128], [1, 100]])
                    nc.gpsimd.dma_start(out=xr[:], in_=src)
                    # im partitions kx-major: p = kx*9 + ky (w rows match on host).
                    # im[p] = xr[ky] shifted left by kx: one CONTIGUOUS 12800-elem
                    # copy per kx (shift in src offset) -> 9 fat descriptors/DMA.
                    im = conv_in.tile([81, 128 * 100], BF16, name="im", tag="im")
                    for kx in range(KW):
                        n = 12800 - kx
                        im_dst = ap(im[:], kx * 9 * 12800, [[12800, 9], [1, n]])
                        im_src = ap(xr[:], kx, [[12800, KH], [1, n]])
                        eng = nc.sync if kx % 2 == 0 else nc.scalar
                        eng.dma_start(out=im_dst, in_=im_src)
                    for oyp in range(3):
                        oy = half * 3 + oyp
                        for oxp in range(WOUT // 2):
                            pos = oy * WOUT + 2 * oxp
                            ps = conv_ps.tile([128, 2 * CONO], F32, name="cps", tag="cps")
                            for j in range(2):
                                lhsT = ap(im[:], 40 * oyp + 2 * (2 * oxp + j),
                                          [[12800, 81], [100, 128]])
                                nc.tensor.matmul(ps[:, j * CONO:(j + 1) * CONO],
                                                 lhsT, w_bf[:], start=True, stop=True)
                            # evac psum [plane, (2 pos, co, no)] -> votes
                            dst = ap(votes[c][:], pos * CONO, [[CHW, 128], [1, 2 * CONO]])
                            if oxp % 2 == 0:
                                nc.vector.tensor_copy(out=dst, in_=ps[:])
                            else:
                                nc.scalar.copy(out=dst, in_=ps[:])

        # ================= ROUTING =================
        rt_ps = ctx.enter_context(tc.tile_pool(name="rt_ps", bufs=2, space="PSUM"))
        work = ctx.enter_context(tc.tile_pool(name="work", bufs=1))
        small = ctx.enter_context(tc.tile_pool(name="small", bufs=1))
        for t in range(ITERS):
            if t == 0:
                red_src = votes
            else:
                # softmax over co per chunk + route*votes
                red_src = []
                for c in range(NCHUNK):
                    # logits are bounded (|logits| < ~4 here), skip max-sub
                    route = small.tile([128, HWCO], BF16, name="route", tag="route")
                    nc.scalar.activation(out=route[:], in_=logits[c][:],
                                         func=Act.Exp, scale=1.0)
                    zs = small.tile([128, POS], F32, name="zs", tag="zs")
                    r_v = ap(route[:], 0, [[HWCO, 128], [CO, POS], [1, CO]])
                    nc.vector.reduce_sum(out=zs[:], in_=r_v, axis=mybir.AxisListType.X)
                    rz = small.tile([128, POS], F32, name="rz", tag="rz")
                    nc.vector.reciprocal(out=rz[:], in_=zs[:])
                    rzb = small.tile([128, POS], BF16, name="rzb", tag="rzb")
                    nc.scalar.copy(out=rzb[:], in_=rz[:])
                    rz_b = ap(rzb[:], 0, [[POS, 128], [1, POS], [0, CO]])
                    nc.vector.tensor_tensor(route[:], route[:], rz_b, Alu.mult)
                    # mult_r = votes * route (broadcast over no)
                    mr = work.tile([128, CHW], BF16, name="mr", tag=f"mr{c % 2}")
                    r_b = ap(route[:], 0, [[HWCO, 128], [CO, POS], [1, CO], [0, NO]])
                    nc.vector.tensor_tensor(mr[:], votes[c][:], r_b, Alu.mult)
                    red_src.append(mr)

            # preact[b][hw, (co,no)] = sum_ci red_src: N=1 matmuls,
            # lhsT = red_src slice [128, hw(36) strided], rhs = ones.
            pre_ps = []
            for b in range(BSH):
                pp = rt_ps.tile([POS, CONO], F32, name="pp", tag=f"pp{b}")
                pre_ps.append(pp)
                for cono in range(CONO):
                    for k, c in enumerate((2 * b, 2 * b + 1)):
                        lhsT = ap(red_src[c][:], cono, [[CHW, 128], [CONO, POS]])
                        nc.tensor.matmul(
                            pp[:, cono:cono + 1], lhsT, ones_bf,
                            start=(k == 0), stop=(k == 1),
                        )

            acts = []
            for b in range(BSH):
                pp = pre_ps[b]
                # preb = pp * (1/CO for iter 0) + bias
                preb = small.tile([POS, CONO], F32, name="preb", tag=f"preb{b}")
                nc.vector.scalar_tensor_tensor(
                    preb[:], pp[:], (1.0 / CO) if t == 0 else 1.0, bias_bc[:],
                    Alu.mult, Alu.add)
                # squash over no
                sq = small.tile([POS, CONO], F32, name="sq", tag=f"sq{b}")
                nc.scalar.activation(out=sq[:], in_=preb[:], func=Act.Square, scale=1.0)
                s2 = small.tile([POS, CO], F32, name="s2", tag=f"s2{b}")
                sq_v = ap(sq[:], 0, [[CONO, POS], [NO, CO], [1, NO]])
                nc.vector.reduce_sum(out=s2[:], in_=sq_v, axis=mybir.AxisListType.X)
                nrm = small.tile([POS, CO], F32, name="nrm", tag=f"nrm{b}")
                nc.scalar.activation(out=nrm[:], in_=s2[:], func=Act.Sqrt, scale=1.0)
                d1 = small.tile([POS, CO], F32, name="d1", tag=f"d1{b}")
                nc.vector.tensor_scalar_add(d1[:], s2[:], 1.0)
                r1 = small.tile([POS, CO], F32, name="r1", tag=f"r1{b}")
                nc.vector.reciprocal(out=r1[:], in_=d1[:])
                fac = small.tile([POS, CO], F32, name="fac", tag=f"fac{b}")
                nc.vector.tensor_tensor(fac[:], nrm[:], r1[:], Alu.mult)
                fac_b = ap(fac[:], 0, [[CO, POS], [1, CO], [0, NO]])
                if t == ITERS - 1:
                    af = small.tile([POS, CONO], F32, name="af", tag=f"af{b}")
                    nc.vector.tensor_tensor(af[:], preb[:], fac_b, Alu.mult)
                    nc.sync.dma_start(
                        out=ap(out_d, b * POS * CONO, [[CONO, POS], [1, CONO]]),
                        in_=af[:],
                    )
                else:
                    ab = small.tile([POS, CONO], BF16, name="ab", tag=f"ab{b}")
                    nc.vector.tensor_tensor(ab[:], preb[:], fac_b, Alu.mult)
                    acts.append(ab)

            if t == ITERS - 1:
                break

            # distances + logits update
            for b in range(BSH):
                # act [POS part, CONO] -> dram bounce -> bcast [128, (hw,co,no)]
                adr = dram.tile([POS, CONO], BF16, name="adr", tag="adr")
                nc.gpsimd.dma_start(out=adr[:], in_=acts[b][:])
                abc = work.tile([128, CHW], BF16, name="abc", tag=f"abc{b}")
                nc.gpsimd.dma_start(out=abc[:], in_=ap(adr[:], 0, [[0, 128], [1, CHW]]))
                for c in (2 * b, 2 * b + 1):
                    md = work.tile([128, CHW], BF16, name="md", tag="md")
                    nc.vector.tensor_tensor(md[:], votes[c][:], abc[:], Alu.mult)
                    dist = small.tile([128, HWCO], F32, name="dist", tag="dist")
                    md_v = ap(md[:], 0, [[CHW, 128], [NO, HWCO], [1, NO]])
                    nc.vector.reduce_sum(out=dist[:], in_=md_v, axis=mybir.AxisListType.X)
                    nc.vector.tensor_tensor(logits[c][:], logits[c][:], dist[:], Alu.add)


@functools.cache
def _program():
    return build_program()


def kernel(x, W, bias, **_ignored):
    x = np.asarray(x, dtype=np.float32)
    W = np.asarray(W, dtype=np.float32)
    bias = np.asarray(bias, dtype=np.float32)
    nc = _program()
    # [81, 256], rows kx-major: r = kx*9 + ky
    w_flat = np.ascontiguousarray(
        W.reshape(CONO, KH, KW).transpose(2, 1, 0).reshape(KH * KW, CONO))
    b_flat = np.ascontiguousarray(bias.reshape(CONO))
    in_maps = []
    for i in range(NCORES):
        xs = x[i * BSH:(i + 1) * BSH].reshape(PLANES, HI * WI)
        in_maps.append({
            "x": np.ascontiguousarray(xs),
            "w": w_flat,
            "b": b_flat,
        })
    res = run_bass_kernel_spmd(nc, in_maps, list(range(NCORES)))
    outs = []
    for i in range(NCORES):
        o = res.results[i]["out"].reshape(BSH, POS, CO, NO)
        outs.append(np.transpose(o, (0, 2, 3, 1)).reshape(BSH, CO, NO, HOUT, WOUT))
    return np.ascontiguousarray(np.concatenate(outs, axis=0))


if __name__ == "__main__":
    xs = np.random.randn(BS, CI, 1, HI, WI).astype(np.float32)
    ws = (np.random.randn(CONO, 1, KH, KW) * 0.05).astype(np.float32)
    bs_ = (np.random.randn(CO, NO, 1, 1) * 0.01).astype(np.float32)
    y = kernel(xs, ws, bs_, quantization_bits=8, quantization_bits_routing=8)
    print(y.shape, y.dtype)

